# revision 1
# baseline (speedup 1.0000x reference)
"""Sliding-window causal attention with RoPE, distributed over 8 NeuronCores.

Sharding: 8 cores = (batch b in {0,1}) x (head-group g in {0..3}); each core
computes its batch's attention for 4 heads (256 channels) plus that group's
partial of the output projection; the host sums the 4 partials per batch.

Pipeline (vs the fp32r baseline at 70.8us):
- Q/K projections are fp8e4 DoubleRow matmuls: 2 D-slices contract per
  instruction at 0.5 cycles/row (4x the fp32r rate). Host pre-scales x by 16
  and wq/wk by 64 so quantization stays in e4m3 normals; the 1/2^20 folds
  into the exp scale. V and output projections stay bf16: fp8 error there
  lands directly in the output (2.4e-2 > tol), while q/k fp8 error mostly
  cancels in softmax renormalization (adds ~9e-3).
- RoPE runs per (chunk, q|k) on m-paired [128,1024] psum spans: two psum
  mults + stream_shuffle + a bf16 2x-mode add on DVE, writing bf16 qr/kr.
- Scores use a 512-col [hi-q2 | mid | lo] layout: one 1-bank psum tile, one
  512-col exp on ACT, banded mask as 3-4 gpsimd affine_selects (post-exp,
  multiplicative), then PV with the ones-augment (rows 64:128 = den sums).
- Softmax normalization: one ACT copy psum->bf16, then reciprocal + two
  2x-mode mults on DVE; the odd head-half reaches its partition slot via a
  small SBUF-SBUF DMA. Output projection accumulates K=256 over both head
  pairs; psum->fp16 via one [128,1024] ACT copy per 128 tokens.
- Projections for chunk c and attention for q-chunks 2c-2, 2c-1 share one
  tile-pool scope per repeat, so DVE-heavy rope overlaps ACT-heavy attention
  (engine busy: PE 47.4us, ACT ~51, DVE ~49, Pool ~34 vs 71/49/63/43 before).

Engine-busy ceiling ~50us/rep; measured repeat-marginal on this box 61.7us
(vs 70.8 baseline, same methodology).
"""

import numpy as np
import ml_dtypes

B, T, D = 2, 2048, 1024
H, HD = 16, 64
G = 4            # head groups (cores per batch)
HPG = H // G     # heads per group = 4
C = HPG * HD     # channels per group = 256
SCALE = 0.125
W = 128          # window per side
NQ = 256         # query chunk
NCHUNK = T // NQ
KT = T // 128    # k tiles
XS, WS = 16.0, 64.0
EXPSCALE = SCALE / (XS * XS * WS * WS)

E4 = ml_dtypes.float8_e4m3
BFNP = ml_dtypes.bfloat16

_cache = {}


def _chan_perm():
    # within-head permutation: pair i=(2i,2i+1) -> block layout where rows
    # [0:16)=re(0..15), [16:32)=im(0..15), [32:48)=re(16..31), [48:64)=im(16..31)
    perm = np.zeros(HD, dtype=np.int64)
    for j in range(HD):
        if j < 16:
            perm[j] = 2 * j
        elif j < 32:
            perm[j] = 2 * (j - 16) + 1
        elif j < 48:
            perm[j] = 2 * (j - 16)
        else:
            perm[j] = 2 * (j - 32) + 1
    return perm


def _pair_of(j):
    return (j % 16) + 16 * (j // 32)


def _build_program(repeat=1):
    import concourse.mybir as mybir
    import concourse.tile as tile
    from concourse import bacc

    F32 = mybir.dt.float32
    F8 = mybir.dt.float8e4
    BF16 = mybir.dt.bfloat16
    FP16 = mybir.dt.float16
    MULT = mybir.AluOpType.mult
    ADD = mybir.AluOpType.add
    DIVOP = mybir.AluOpType.divide
    EXP = mybir.ActivationFunctionType.Exp
    DR = mybir.MatmulPerfMode.DoubleRow

    nc = bacc.Bacc("TRN2", target_bir_lowering=False, debug=False, num_devices=8)

    x8T = nc.dram_tensor("x8T", (D, T), F8, kind="ExternalInput")
    xbT = nc.dram_tensor("xbT", (D, T), BF16, kind="ExternalInput")
    wq8T = nc.dram_tensor("wq8T", (D, C), F8, kind="ExternalInput")
    wk8T = nc.dram_tensor("wk8T", (D, C), F8, kind="ExternalInput")
    wvT = nc.dram_tensor("wvT", (D, C), BF16, kind="ExternalInput")
    woT = nc.dram_tensor("woT", (C, D), BF16, kind="ExternalInput")
    cosT = nc.dram_tensor("cosT", (128, T), BF16, kind="ExternalInput")
    sinT = nc.dram_tensor("sinT", (128, T), BF16, kind="ExternalInput")
    out = nc.dram_tensor("out", (T, D), FP16, kind="ExternalOutput")

    shuf16 = [(i + 16) % 32 for i in range(32)]

    with tile.TileContext(nc) as tc:
        with tc.tile_pool(name="persist", bufs=1) as pp:
            # ---- loads: first projection chunk's operands first
            wq8_a = pp.tile([128, 8, C], F8, tag="wq8_a", name="wq8_a")
            x8_t = []
            x8_t.append(pp.tile([128, 8, 512], F8, tag="x8_0", name="x8_0"))
            nc.sync.dma_start(x8_t[0][:], x8T[:, 0:512].rearrange("(g p) t -> p g t", p=128))
            nc.sync.dma_start(wq8_a[:], wq8T.rearrange("(g p) c -> p g c", p=128))
            wk8_a = pp.tile([128, 8, C], F8, tag="wk8_a", name="wk8_a")
            nc.sync.dma_start(wk8_a[:], wk8T.rearrange("(g p) c -> p g c", p=128))
            cos_c, sin_c = [], []
            for c in range(4):
                cc = pp.tile([128, 1024], BF16, tag=f"cos{c}", name=f"cos{c}")
                sc_ = pp.tile([128, 1024], BF16, tag=f"sin{c}", name=f"sin{c}")
                if c == 0:
                    for m in range(2):
                        nc.sync.dma_start(cc[:, m * 512:(m + 1) * 512], cosT[:, 0:512])
                        nc.sync.dma_start(sc_[:, m * 512:(m + 1) * 512], sinT[:, 0:512])
                cos_c.append(cc)
                sin_c.append(sc_)
            for c in range(1, 4):
                x8_t.append(pp.tile([128, 8, 512], F8, tag=f"x8_{c}", name=f"x8_{c}"))
                nc.sync.dma_start(x8_t[c][:],
                                  x8T[:, c * 512:(c + 1) * 512].rearrange("(g p) t -> p g t", p=128))
                for m in range(2):
                    nc.sync.dma_start(cos_c[c][:, m * 512:(m + 1) * 512],
                                      cosT[:, c * 512:(c + 1) * 512])
                    nc.sync.dma_start(sin_c[c][:, m * 512:(m + 1) * 512],
                                      sinT[:, c * 512:(c + 1) * 512])
            wv_a = pp.tile([128, 8, C], BF16, tag="wv_a", name="wv_a")
            nc.sync.dma_start(wv_a[:], wvT.rearrange("(g p) c -> p g c", p=128))
            wo_a = pp.tile([128, 2, D], BF16, tag="wo_a", name="wo_a")
            nc.sync.dma_start(wo_a[:], woT.rearrange("(g p) c -> p g c", p=128))
            xb_t = {}
            for c in range(4):
                for k in range(8):
                    xb_t[k, c] = pp.tile([128, 512], BF16, tag=f"xb{k}_{c}", name=f"xb{k}_{c}")
                    nc.sync.dma_start(xb_t[k, c][:], xbT[k * 128:(k + 1) * 128,
                                                        c * 512:(c + 1) * 512])

            # ---- persistent activation storage
            qr_c = [pp.tile([128, 1024], BF16, tag=f"qr{c}", name=f"qr{c}") for c in range(4)]
            kr_c = [pp.tile([128, 1024], BF16, tag=f"kr{c}", name=f"kr{c}") for c in range(4)]
            # v tiles: vt2[i] holds k-tiles 2i, 2i+1; per head 128 cols [ch 64 | ones 64]
            vt2 = [pp.tile([128, 1024], BF16, tag=f"v{t}", name=f"v{t}") for t in range(KT // 2)]
            for t in range(KT // 2):
                nc.gpsimd.memset(vt2[t][:], 1.0)

            for _rep in range(repeat):
                # ==== fused projections + attention: attention for q-chunks
                # (2c, 2c+1) issues right after chunk c's projections, so the
                # DVE-heavy rope work of chunk c+1 overlaps the ACT/PE-heavy
                # attention of chunk c.
                with tc.tile_pool(name="projps", bufs=1, space="PSUM") as pjp, \
                     tc.tile_pool(name="smps", bufs=2, space="PSUM") as pjv, \
                     tc.tile_pool(name="otps", bufs=1, space="PSUM") as otp, \
                     tc.tile_pool(name="wops", bufs=1, space="PSUM") as wop, \
                     tc.tile_pool(name="ropetmp", bufs=8) as rtp, \
                     tc.tile_pool(name="attsb", bufs=6) as asb, \
                     tc.tile_pool(name="outsb", bufs=4) as osb:
                    for c in range(5):
                      if c < 4:
                        for (w8a, dst) in ((wq8_a, qr_c), (wk8_a, kr_c)):
                            ps = pjp.tile([128, 1024], F32, tag="proj", name="proj")
                            for m in range(2):
                                for j in range(4):
                                    nc.tensor.matmul(ps[:, m * 512:(m + 1) * 512],
                                                     w8a[:, 2 * j:2 * j + 2, m * 128:(m + 1) * 128],
                                                     x8_t[c][:, 2 * j:2 * j + 2, :],
                                                     start=(j == 0), stop=(j == 3),
                                                     perf_mode=DR)
                            z = rtp.tile([128, 1024], BF16, tag="ropez", name="ropez")
                            nc.vector.tensor_tensor(z[:], ps[:], sin_c[c][:], MULT)
                            zs = rtp.tile([128, 1024], BF16, tag="ropezs", name="ropezs")
                            nc.vector.stream_shuffle(zs[:], z[:], shuf16)
                            t1 = rtp.tile([128, 1024], BF16, tag="ropet1", name="ropet1")
                            nc.vector.tensor_tensor(t1[:], ps[:], cos_c[c][:], MULT)
                            nc.vector.tensor_tensor(dst[c][:], t1[:], zs[:], ADD)
                        # v projection: 2 token-tiles per psum tile
                        for half in range(2):
                            psv = pjv.tile([128, 512], F32, tag="sm", name="sm")
                            for loc in range(2):
                                tloc = 2 * half + loc
                                for k in range(8):
                                    nc.tensor.matmul(psv[:, loc * 256:(loc + 1) * 256],
                                                     xb_t[k, c][:, tloc * 128:tloc * 128 + 128],
                                                     wv_a[:, k, :],
                                                     start=(k == 0), stop=(k == 7))
                            src_ = psv[:].rearrange("p (t h d) -> p t h d", t=2, h=HPG)
                            dstv = vt2[c * 2 + half][:].rearrange(
                                "p (t h d) -> p t h d", t=2, h=HPG)[:, :, :, 0:64]
                            nc.scalar.copy(dstv, src_)

                      # software pipeline: attention lags projections by one
                      # chunk so DVE rope(c) never waits on att-scale deps
                      if True:
                        for qc in ((2 * c - 2, 2 * c - 1) if c > 0 else ()):
                            cq = qc // 2
                            qcol = (qc % 2) * 256
                            # sc layout (512 cols): [hi-q2 0:128 | mid 128:384 | lo 384:512]
                            # hi tile = 2qc+1, mid = 2qc, lo = 2qc-1
                            have_lo = qc > 0
                            ehi = 512 if have_lo else 384
                            stacked = [asb.tile([128, 256], BF16, tag=f"stk{p}", name=f"stk{p}")
                                       for p in range(2)]
                            pair_ots = [None, None]
                            for h in range(HPG):
                                m, hh = h // 2, h % 2
                                hp = slice(64 * hh, 64 * hh + 64)
                                sc = pjv.tile([128, 512], F32, tag="sm", name="sm")
                                kt_hi, kt_mid, kt_lo = 2 * qc + 1, 2 * qc, 2 * qc - 1

                                def krsl(kt, n):
                                    return kr_c[kt // 4][hp, m * 512 + (kt % 4) * 128:
                                                         m * 512 + (kt % 4) * 128 + n]

                                qsl = qr_c[cq][hp, m * 512 + qcol:m * 512 + qcol + 256]
                                qlo = qr_c[cq][hp, m * 512 + qcol:m * 512 + qcol + 128]
                                qhi = qr_c[cq][hp, m * 512 + qcol + 128:m * 512 + qcol + 256]
                                nc.tensor.matmul(sc[:, 0:128], krsl(kt_hi, 128), qhi,
                                                 start=True, stop=True)
                                nc.tensor.matmul(sc[:, 128:384], krsl(kt_mid, 128), qsl,
                                                 start=True, stop=True)
                                if have_lo:
                                    nc.tensor.matmul(sc[:, 384:512], krsl(kt_lo, 128), qlo,
                                                     start=True, stop=True)
                                probs = asb.tile([128, 512], BF16, tag="probs", name="probs")
                                nc.scalar.activation(probs[:, 0:ehi], sc[:, 0:ehi],
                                                     EXP, bias=0.0, scale=EXPSCALE)
                                # banded mask, multiplicative post-exp on Pool:
                                # [0:128) hi and [128:256) mid-q1 keep col >= p;
                                # [256:384) mid-q2 and [384:512) lo keep p >= col
                                GE = mybir.AluOpType.is_ge
                                # cyclic [[0,2],[+-1,128]] patterns apply the
                                # same 128-col triangle to two adjacent regions
                                # in ONE select each (hi+mid-q1, mid-q2[+lo])
                                nc.gpsimd.affine_select(probs[:, 0:256], probs[:, 0:256],
                                                        pattern=[[0, 2], [1, 128]], compare_op=GE,
                                                        fill=0.0, base=0, channel_multiplier=-1)
                                lsz = 256 if have_lo else 128
                                nc.gpsimd.affine_select(probs[:, 256:256 + lsz], probs[:, 256:256 + lsz],
                                                        pattern=[[0, lsz // 128], [-1, 128]], compare_op=GE,
                                                        fill=0.0, base=0, channel_multiplier=1)
                                # PV with ones-augment: rows 0:64 oT_h, rows 64:128 sums
                                if h == 0:
                                    pair_ots[0] = otp.tile([128, 1024], F32, tag="ot", name="ot")
                                    pair_ots[1] = pair_ots[0]
                                po = pair_ots[0][:, m * 512:(m + 1) * 512]
                                ot = po[:, hh * 256:(hh + 1) * 256]
                                def vsl(kt):
                                    o = (kt % 2) * 512 + h * 128
                                    return vt2[kt // 2][:, o:o + 128]

                                nc.tensor.matmul(ot, vsl(kt_mid), probs[:, 128:384],
                                                 start=True, stop=False)
                                nc.tensor.matmul(po[:, hh * 256 + 128:(hh + 1) * 256],
                                                 vsl(kt_hi), probs[:, 0:128],
                                                 start=False, stop=(not have_lo))
                                if have_lo:
                                    nc.tensor.matmul(po[:, hh * 256:hh * 256 + 128],
                                                     vsl(kt_lo), probs[:, 384:512],
                                                     start=False, stop=True)
                                if h == 3:
                                    # normalize BOTH m-pairs at once: one psum->
                                    # bf16 stage copy (ACT) + one reciprocal
                                    # (DVE), then four 2x-mode mults whose output
                                    # windows base-shift directly (no DMA)
                                    potsb = asb.tile([128, 1024], BF16, tag="potsb", name="potsb")
                                    nc.scalar.copy(potsb[:], pair_ots[0][:])
                                    rbc = asb.tile([64, 1024], BF16, tag="rbc", name="rbc")
                                    with nc.allow_low_precision(reason="bf16 softmax denom"):
                                        nc.vector.reciprocal(rbc[:], potsb[64:128, :])
                                    for mm in range(2):
                                        o = mm * 512
                                        nc.vector.tensor_tensor(stacked[mm][0:64, :],
                                                                potsb[0:64, o:o + 256],
                                                                rbc[:, o:o + 256], MULT)
                                        nc.vector.tensor_tensor(stacked[mm][64:128, :],
                                                                potsb[0:64, o + 256:o + 512],
                                                                rbc[:, o + 256:o + 512], MULT)
                            # output projection for this q-chunk
                            for tc2 in range(2):
                                trows = qc * 256 + tc2 * 128
                                wps2 = wop.tile([128, 1024], F32, tag="wps", name="wps")
                                for nh in range(2):
                                    for p in range(2):
                                        nc.tensor.matmul(wps2[:, nh * 512:(nh + 1) * 512],
                                                         stacked[p][:, tc2 * 128:tc2 * 128 + 128],
                                                         wo_a[:, p, nh * 512:(nh + 1) * 512],
                                                         start=(p == 0), stop=(p == 1))
                                ob = osb.tile([128, 1024], FP16, tag="ob", name="ob")
                                nc.scalar.copy(ob[:], wps2[:])
                                nc.sync.dma_start(out[trows:trows + 128, :], ob[:])

    nc.compile()
    return nc


def _prep_inputs(x, rope_cos, rope_sin, wq, wk, wv, wo):
    perm = _chan_perm()
    pairs = np.array([_pair_of(j) for j in range(HD)])
    sgn = np.where((np.arange(HD) % 32) < 16, 1.0, -1.0).astype(np.float32)

    # (128, T) rope tiles in de-interleaved layout; identical for both 2-head tiles
    j64 = np.arange(128) % HD
    cos_t = np.ascontiguousarray(rope_cos.T[pairs[j64], :]).astype(BFNP)
    sin_t = np.ascontiguousarray(
        (rope_sin.T[pairs[j64], :] * sgn[j64][:, None])).astype(BFNP)

    def q8(a):
        return np.clip(a, -240.0, 240.0).astype(E4)

    ins = []
    for b in range(B):
        xTb = np.ascontiguousarray(x[b].T)                          # (D, T)
        x8Tb = q8(xTb * XS)
        xbTb = xTb.astype(BFNP)
        for g in range(G):
            rows = np.concatenate([g * C + h * HD + perm for h in range(HPG)])
            wq8g = q8(np.ascontiguousarray(wq[rows, :].T) * WS)      # (D, C)
            wk8g = q8(np.ascontiguousarray(wk[rows, :].T) * WS)
            wvTg = np.ascontiguousarray(wv[g * C:(g + 1) * C, :].T).astype(BFNP)
            woTg = np.ascontiguousarray(wo[:, g * C:(g + 1) * C].T).astype(BFNP)
            ins.append({
                "x8T": x8Tb, "xbT": xbTb, "wq8T": wq8g, "wk8T": wk8g,
                "wvT": wvTg, "woT": woTg,
                "cosT": cos_t, "sinT": sin_t,
            })
    return ins


def kernel(x, rope_cos, rope_sin, wq, wk, wv, wo, _trace=False):
    from concourse.bass_utils import run_bass_kernel_spmd

    if "nc" not in _cache:
        _cache["nc"] = _build_program()
    nc = _cache["nc"]

    ins = _prep_inputs(np.asarray(x, np.float32), np.asarray(rope_cos, np.float32),
                       np.asarray(rope_sin, np.float32), np.asarray(wq, np.float32),
                       np.asarray(wk, np.float32), np.asarray(wv, np.float32),
                       np.asarray(wo, np.float32))
    kwargs = {}
    if _trace:
        kwargs = dict(trace=True)
    res = run_bass_kernel_spmd(nc, ins, core_ids=list(range(8)), **kwargs)
    _cache["last_result"] = res

    out = np.zeros((B, T, D), dtype=np.float32)
    for i in range(8):
        out[i // G] += res.results[i]["out"].astype(np.float32)
    return out

